# revision 2
# baseline (speedup 1.0000x reference)
"""GroupedQueryAttention (B=2, S=2048, DIM=1024, 16 heads, 4 KV groups) on 8 trn2 cores.

Wall-clock through the axon tunnel is dominated by host<->device bytes and
per-transfer overhead, so:
- core c owns heads {2c, 2c+1} (kv group g = c//2) for BOTH batches.
- ALL per-core inputs ship as ONE packed bf16-typed tensor [578, 1024]:
  rows 0:256    x slice (rows c*512:(c+1)*512 of b-major flattened x),
                int8-quantized per row, bitcast into bf16 storage
  row  256      the 512 per-row f32 x scales, bitcast
  rows 257:321  this core's HALF of the blocked [w_k|w_v] group weights
  rows 321:449  wq blocked (this core's 2 head columns)
  rows 449:577  wo slice (this core's 2 head rows)
  row  577      the two f32 bias vectors (beta @ w), bitcast
- the x slices are AllGathered on-device (compact int8+scales form) and
  dequantized per 128-row tile; the kv-weight halves are AllGathered over
  core pairs {2g, 2g+1}.
- partial out-projections are summed with an on-device ReduceScatter(add) so
  core c keeps only rows c*512:(c+1)*512 of y, which it emits int8-quantized
  with per-row f32 scales (bitcast into two extra int8 rows) -> 0.5MB output
  per core; host dequantizes + adds b_o.

Device math: LN stats f32, matmuls bf16 x bf16 -> f32 PSUM, softmax
denominator via ones-column in the PV matmul, normalization deferred to the
[64, s_q] head outputs.
"""

import numpy as np
import ml_dtypes

import concourse.bass as bass
import concourse.mybir as mybir
from concourse import bacc
from concourse.bass_utils import run_bass_kernel_spmd
from concourse.tile import TileContext
from concourse.masks import make_identity

B, S, DIM = 2, 2048, 1024
HEADS, DH, G = 16, 64, 4
SCALE = DH ** -0.5
P = 128
NCORES = 8
RS = 512                      # x rows / y rows per core
SALL = B * S                  # 4096 rows total (b-major)
NT_R = SALL // P              # 32 row tiles
NT_D = DIM // P               # 8 contraction chunks
XCH = RS // 2 + 1             # 257 pack rows per x chunk (256 int8 + 1 scales)
PACK_ROWS = XCH + 64 + P + P + 1   # 578
R_WKV = XCH                   # 257
R_WQ = XCH + 64               # 321
R_WO = R_WQ + P               # 449
R_BIAS = R_WO + P             # 577
F32 = mybir.dt.float32
BF16 = mybir.dt.bfloat16
INT8 = mybir.dt.int8
NP_BF16 = ml_dtypes.bfloat16
AF = mybir.ActivationFunctionType
OP = mybir.AluOpType
AX = mybir.AxisListType
ALL_CORES = [list(range(NCORES))]
PAIRS = [[2 * g, 2 * g + 1] for g in range(G)]


def build_nc():
    nc = bacc.Bacc("TRN2", target_bir_lowering=False, num_devices=NCORES)
    pack = nc.dram_tensor("pack", [PACK_ROWS, DIM], BF16, kind="ExternalInput")
    y = nc.dram_tensor("y", [RS + 2, DIM], INT8, kind="ExternalOutput")

    qb_d = pack[R_BIAS:R_BIAS + 1, 0:2 * P].bitcast(F32)       # [1, 128]
    kvb_d = pack[R_BIAS:R_BIAS + 1, 2 * P:4 * P].bitcast(F32)  # [1, 128]

    with TileContext(nc) as tc:
        with tc.tile_pool(name="dram", bufs=1, space="DRAM") as dp, \
             tc.tile_pool(name="persist", bufs=1) as pp:
            # Collective buffers with chunks >256KB must sit at MB-aligned pool
            # offsets (misaligned multi-hop ring chunks corrupt beyond 256KB):
            # declare big tiles first, carve small buffers out of a 1MB slot.
            # NB: the collective forwarding path flushes bf16 denormals to
            # zero on multi-hop chunks, so raw-byte payloads (int8 x data,
            # bitcast f32 scales) MUST travel in int8-typed buffers.
            xall8 = dp.tile([NCORES * 256, 2 * DIM], INT8)    # 4MB @0
            ypart = dp.tile([SALL, DIM], BF16)                # 8MB @4MB
            yscat = dp.tile([RS, DIM], BF16)                  # 1MB @12MB
            xb8 = dp.tile([256, 2 * DIM], INT8)               # 512KB @13MB
            sb8 = dp.tile([1, 2 * DIM], INT8)
            sall8 = dp.tile([NCORES, 2 * DIM], INT8)
            wkv_bounce = dp.tile([64, DIM], BF16)
            wkv_full = dp.tile([P, DIM], BF16)

            nc.gpsimd.dma_start(xb8[:], pack[0:RS // 2, :].bitcast(INT8))
            nc.gpsimd.collective_compute(
                "AllGather", OP.bypass, replica_groups=ALL_CORES,
                ins=[xb8[:].opt()], outs=[xall8[:].opt()])
            nc.gpsimd.dma_start(sb8[:], pack[RS // 2:RS // 2 + 1, :].bitcast(INT8))
            nc.gpsimd.collective_compute(
                "AllGather", OP.bypass, replica_groups=ALL_CORES,
                ins=[sb8[:].opt()], outs=[sall8[:].opt()])
            nc.gpsimd.dma_start(wkv_bounce[:], pack[R_WKV:R_WKV + 64, :])
            nc.gpsimd.collective_compute(
                "AllGather", OP.bypass, replica_groups=PAIRS,
                ins=[wkv_bounce[:].opt()], outs=[wkv_full[:].opt()])

            ident = pp.tile([P, P], F32)
            make_identity(nc, ident[:])
            identb = pp.tile([P, P], BF16)
            nc.vector.tensor_copy(out=identb[:], in_=ident[:])
            wq_sb = pp.tile([P, DIM], BF16)    # [p, c*128+j] = wq[c*128+p, j]
            wkv_sb = pp.tile([P, DIM], BF16)
            wo_sb = pp.tile([P, DIM], BF16)
            qb_sb = pp.tile([P, 1], F32)
            kvb_sb = pp.tile([P, 1], F32)
            nc.sync.dma_start(out=wq_sb[:], in_=pack[R_WQ:R_WQ + P, :])
            nc.sync.dma_start(out=wkv_sb[:], in_=wkv_full[:])
            nc.sync.dma_start(out=wo_sb[:], in_=pack[R_WO:R_WO + P, :])
            nc.sync.dma_start(out=qb_sb[:], in_=qb_d.rearrange("a p -> p a"))
            nc.sync.dma_start(out=kvb_sb[:], in_=kvb_d.rearrange("a p -> p a"))

            eps_sb = pp.tile([P, 1], F32)
            nc.vector.memset(eps_sb[:], 1e-5)

            qT = pp.tile([P, SALL], BF16)      # rows 0:64 head-even, 64:128 head-odd
            kvT = pp.tile([P, SALL], BF16)     # rows 0:64 = kT, 64:128 = vT
            kdup = pp.tile([P, SALL], BF16)    # rows 64:128 = kT copy (odd-head lhsT)
            vones = pp.tile([P, NT_R, DH + 1], BF16)
            outT = pp.tile([P, SALL], BF16)

            # ---------- Phase 1: dequant + LayerNorm + transpose ----------
            with tc.tile_pool(name="xnTp", bufs=1) as xp:
                xnT = xp.tile([P, NT_D, SALL], BF16)
                with tc.tile_pool(name="ln", bufs=2) as lnp, \
                     tc.tile_pool(name="scr", bufs=2) as scp, \
                     tc.tile_pool(name="lns", bufs=2) as lsp, \
                     tc.tile_pool(name="psT", bufs=2, space="PSUM") as ptp, \
                     tc.tile_pool(name="psP", bufs=3, space="PSUM") as ppp:
                    for i in range(NT_R):
                        r, t = i // 4, i % 4
                        x8 = lnp.tile([P, DIM], INT8, tag="x8")
                        nc.sync.dma_start(
                            out=x8[:],
                            in_=xall8[r * 256 + t * 64:r * 256 + (t + 1) * 64, :]
                                .rearrange("a (b c) -> (a b) c", c=DIM))
                        xsc = lsp.tile([P, 1], F32, tag="xsc")
                        nc.sync.dma_start(
                            out=xsc[:],
                            in_=sall8[r:r + 1, t * 512:(t + 1) * 512]
                                .bitcast(F32).rearrange("a p -> p a"))
                        xt = lnp.tile([P, DIM], BF16, tag="x")
                        nc.vector.tensor_scalar_mul(xt[:], x8[:], xsc[:])
                        sm = lsp.tile([P, 1], F32, tag="sm")
                        nc.vector.tensor_reduce(out=sm[:], in_=xt[:], axis=AX.X, op=OP.add)
                        scr = scp.tile([P, DIM], F32, tag="scr")
                        ssq = lsp.tile([P, 1], F32, tag="ssq")
                        nc.scalar.activation(scr[:], xt[:], AF.Square, accum_out=ssq[:])
                        msq = lsp.tile([P, 1], F32, tag="msq")
                        nc.scalar.mul(msq[:], ssq[:], 1.0 / DIM)
                        negmu = lsp.tile([P, 1], F32, tag="negmu")
                        nc.scalar.mul(negmu[:], sm[:], -1.0 / DIM)
                        mu2 = lsp.tile([P, 1], F32, tag="mu2")
                        nc.vector.tensor_mul(mu2[:], negmu[:], negmu[:])
                        var = lsp.tile([P, 1], F32, tag="var")
                        nc.vector.tensor_sub(var[:], msq[:], mu2[:])
                        std = lsp.tile([P, 1], F32, tag="std")
                        nc.scalar.activation(std[:], var[:], AF.Sqrt, bias=eps_sb[:])
                        rstd = lsp.tile([P, 1], F32, tag="rstd")
                        nc.vector.reciprocal(rstd[:], std[:])
                        xn = lnp.tile([P, DIM], BF16, tag="xn")
                        nc.vector.tensor_scalar(
                            out=xn[:], in0=xt[:], scalar1=negmu[:],
                            scalar2=rstd[:], op0=OP.add, op1=OP.mult)
                        pt = ptp.tile([P, DIM], BF16, tag="pt")
                        for j in range(NT_D):
                            nc.tensor.transpose(pt[:, j * P:(j + 1) * P],
                                                xn[:, j * P:(j + 1) * P], identb[:])
                        nc.vector.tensor_copy(
                            out=xnT[:, :, i * P:(i + 1) * P],
                            in_=pt[:].rearrange("p (j c) -> p j c", j=NT_D))

                    # ---------- Phase 2: q / kv projections ----------
                    for rc in range(SALL // 512):
                        r0 = rc * 512
                        pq = ppp.tile([P, 512], F32, tag="pq")
                        for c in range(NT_D):
                            nc.tensor.matmul(
                                pq[:], lhsT=wq_sb[:, c * P:(c + 1) * P],
                                rhs=xnT[:, c, r0:r0 + 512],
                                start=(c == 0), stop=(c == NT_D - 1))
                        nc.vector.tensor_scalar_add(qT[:, r0:r0 + 512], pq[:], qb_sb[:])
                        pkv = ppp.tile([P, 512], F32, tag="pq")
                        for c in range(NT_D):
                            nc.tensor.matmul(
                                pkv[:], lhsT=wkv_sb[:, c * P:(c + 1) * P],
                                rhs=xnT[:, c, r0:r0 + 512],
                                start=(c == 0), stop=(c == NT_D - 1))
                        nc.vector.tensor_scalar_add(kvT[:, r0:r0 + 512], pkv[:], kvb_sb[:])

                    # kT copy to partitions 64:128; V natural layout + ones col
                    nc.sync.dma_start(out=kdup[64:128, :], in_=kvT[0:DH, :])
                    ones_col = pp.tile([P, 1], F32)
                    nc.vector.memset(ones_col[:], 1.0)
                    nc.vector.tensor_copy(out=vones[:, :, DH],
                                          in_=ones_col[:].broadcast_to([P, NT_R]))
                    for m in range(NT_R):
                        pv = ppp.tile([P, DH], BF16, tag="pq")
                        nc.tensor.transpose(pv[:], kvT[64:128, m * P:(m + 1) * P],
                                            identb[64:128, 64:128])
                        nc.vector.tensor_copy(out=vones[:, m, 0:DH], in_=pv[:])

            # ---------- Phase 3: attention (2 heads x 2 batches) ----------
            with tc.tile_pool(name="att", bufs=4) as ap_, \
                 tc.tile_pool(name="bc", bufs=2) as bp, \
                 tc.tile_pool(name="psS", bufs=2, space="PSUM") as psp, \
                 tc.tile_pool(name="psO", bufs=2, space="PSUM") as pop:
                for h in range(2):
                    pr = h * DH
                    kk = kvT if h == 0 else kdup
                    for b in range(B):
                        s0 = b * S
                        po0 = pop.tile([DH + 1, 1024], F32, tag="po")
                        po1 = pop.tile([DH + 1, 1024], F32, tag="po")
                        pos = [po0, po1]
                        for m in range(S // P):
                            ess = []
                            for half in range(2):
                                q0 = s0 + half * 1024
                                ps = psp.tile([P, 1024], F32, tag="ps")
                                for n in range(2):
                                    nc.tensor.matmul(
                                        ps[:, n * 512:(n + 1) * 512],
                                        lhsT=kk[pr:pr + DH, s0 + m * P:s0 + (m + 1) * P],
                                        rhs=qT[pr:pr + DH, q0 + n * 512:q0 + (n + 1) * 512],
                                        start=True, stop=True)
                                es = ap_.tile([P, 1024], BF16, tag="es")
                                nc.scalar.activation(es[:], ps[:], AF.Exp, scale=SCALE)
                                ess.append(es)
                            mg = (s0 + m * P) // P
                            for half in range(2):
                                for n in range(2):
                                    nc.tensor.matmul(
                                        pos[half][:, n * 512:(n + 1) * 512],
                                        lhsT=vones[:, mg, :],
                                        rhs=ess[half][:, n * 512:(n + 1) * 512],
                                        start=(m == 0), stop=(m == S // P - 1))
                        for half in range(2):
                            q0 = s0 + half * 1024
                            po = pos[half]
                            ot = bp.tile([DH + 1, 1024], F32, tag="ot")
                            nc.vector.tensor_copy(out=ot[:], in_=po[:])
                            rc_ = bp.tile([1, 1024], F32, tag="rc")
                            nc.vector.reciprocal(rc_[:], ot[DH:DH + 1, :])
                            rbs = bp.tile([DH, 1024], F32, tag="rbs")
                            nc.gpsimd.partition_broadcast(rbs[:], rc_[:])
                            if h == 0:
                                nc.vector.tensor_mul(
                                    outT[0:DH, q0:q0 + 1024], ot[0:DH, :], rbs[:])
                            else:
                                st = bp.tile([DH, 1024], BF16, tag="st")
                                nc.vector.tensor_mul(st[:], ot[0:DH, :], rbs[:])
                                nc.sync.dma_start(
                                    out=outT[DH:2 * DH, q0:q0 + 1024], in_=st[:])

            # ---------- Phase 4: partial out-projection ----------
            with tc.tile_pool(name="yt", bufs=2) as yp, \
                 tc.tile_pool(name="psY", bufs=2, space="PSUM") as pyp:
                for rc in range(NT_R):
                    py = pyp.tile([P, DIM], F32, tag="py")
                    for n in range(2):
                        nc.tensor.matmul(
                            py[:, n * 512:(n + 1) * 512],
                            lhsT=outT[:, rc * P:(rc + 1) * P],
                            rhs=wo_sb[:, n * 512:(n + 1) * 512],
                            start=True, stop=True)
                    yt = yp.tile([P, DIM], BF16, tag="yt")
                    nc.vector.tensor_copy(out=yt[:], in_=py[:])
                    nc.sync.dma_start(out=ypart[rc * P:(rc + 1) * P, :], in_=yt[:])

            # ---------- Phase 5: ReduceScatter partials; core c keeps rows c*512 ----------
            nc.gpsimd.collective_compute(
                "ReduceScatter", OP.add, replica_groups=ALL_CORES,
                ins=[ypart[:].opt()], outs=[yscat[:].opt()])

            # ---------- Phase 6: int8 quantization with per-row scales ----------
            NT_Y = RS // P  # 4
            with tc.tile_pool(name="qz", bufs=2) as qp, \
                 tc.tile_pool(name="qs", bufs=1) as sp, \
                 tc.tile_pool(name="psQ", bufs=1, space="PSUM") as pqp:
                scs = sp.tile([P, NT_Y], F32)
                for t in range(NT_Y):
                    ysb = qp.tile([P, DIM], BF16, tag="ysb")
                    nc.sync.dma_start(out=ysb[:], in_=yscat[t * P:(t + 1) * P, :])
                    ya = qp.tile([P, DIM], F32, tag="ya")
                    nc.scalar.activation(ya[:], ysb[:], AF.Abs)
                    amax = sp.tile([P, 1], F32)
                    nc.vector.tensor_reduce(out=amax[:], in_=ya[:], axis=AX.X,
                                            op=OP.max)
                    sc = sp.tile([P, 1], F32)
                    nc.scalar.activation(sc[:], amax[:], AF.Copy,
                                         scale=1.0 / 126.5, bias=1e-30)
                    nc.vector.tensor_copy(out=scs[:, t:t + 1], in_=sc[:])
                    rsc = sp.tile([P, 1], F32)
                    nc.vector.reciprocal(rsc[:], sc[:])
                    q8 = qp.tile([P, DIM], INT8, tag="q8")
                    nc.vector.tensor_scalar_mul(q8[:], ysb[:], rsc[:])
                    nc.sync.dma_start(out=y[t * P:(t + 1) * P, :], in_=q8[:])
                # pack scales (row-order f32) into the last two int8 rows
                pst = pqp.tile([NT_Y, P], F32)
                nc.tensor.transpose(pst[:], scs[:], ident[:])
                sct = sp.tile([NT_Y, P], F32)
                nc.vector.tensor_copy(out=sct[:], in_=pst[:])
                nc.sync.dma_start(
                    out=y[RS:RS + 2, :].rearrange("a (b c) -> (a b) c", c=512),
                    in_=sct[:].bitcast(INT8))

    nc.compile()
    return nc


_NC = None


def _get_nc():
    global _NC
    if _NC is None:
        _NC = build_nc()
    return _NC


def _block(w):
    # [1024, N] -> [N, 1024] blocked: [p, c*128+j] = w[c*128+p, j], N cols
    n = w.shape[1]
    return w.reshape(NT_D, P, n).transpose(1, 0, 2).reshape(P, NT_D * n)


def make_in_maps(x, ln_gamma, ln_beta, w_q, w_k, w_v, w_o):
    x = np.asarray(x, np.float32).reshape(SALL, DIM)
    g_ = np.asarray(ln_gamma, np.float32)[:, None]
    b_ = np.asarray(ln_beta, np.float32)
    w_q = np.asarray(w_q, np.float32)
    w_k = np.asarray(w_k, np.float32)
    w_v = np.asarray(w_v, np.float32)
    w_o = np.asarray(w_o, np.float32)
    in_maps = []
    for core in range(NCORES):
        g = core // 2
        wq_s = w_q[:, core * P:(core + 1) * P]
        wkv_s = np.concatenate([w_k[:, g * DH:(g + 1) * DH],
                                w_v[:, g * DH:(g + 1) * DH]], axis=1)
        qkvb = np.stack([b_ @ wq_s, b_ @ wkv_s]).astype(np.float32)
        # x slice -> per-row int8 + f32 scales
        xi = x[core * RS:(core + 1) * RS]
        xsc = (np.abs(xi).max(axis=1) / 126.5 + 1e-30).astype(np.float32)
        x8 = np.clip(np.rint(xi / xsc[:, None]), -127, 127).astype(np.int8)
        pk = np.empty((PACK_ROWS, DIM), NP_BF16)
        pk[0:RS // 2] = np.frombuffer(x8.tobytes(), dtype=NP_BF16).reshape(RS // 2, DIM)
        pk[RS // 2] = np.frombuffer(xsc.tobytes(), dtype=NP_BF16)
        wkv_blk = _block((g_ * wkv_s).astype(NP_BF16))
        half = slice(0, 64) if core % 2 == 0 else slice(64, P)
        pk[R_WKV:R_WKV + 64] = wkv_blk[half]
        pk[R_WQ:R_WQ + P] = _block((g_ * wq_s).astype(NP_BF16))
        pk[R_WO:R_WO + P] = w_o[core * P:(core + 1) * P, :]
        brow = np.zeros((DIM,), NP_BF16)
        brow[0:4 * P] = np.frombuffer(qkvb.tobytes(), dtype=NP_BF16)
        pk[R_BIAS] = brow
        in_maps.append({"pack": pk})
    return in_maps


def kernel(x, ln_gamma, ln_beta, w_q, w_k, w_v, w_o, b_o):
    nc = _get_nc()
    in_maps = make_in_maps(x, ln_gamma, ln_beta, w_q, w_k, w_v, w_o)
    res = run_bass_kernel_spmd(nc, in_maps, list(range(NCORES)))
    out = np.empty((SALL, DIM), np.float32)
    for c in range(NCORES):
        yq = np.asarray(res.results[c]["y"])
        scales = np.frombuffer(yq[RS:RS + 2].tobytes(), dtype=np.float32)
        out[c * RS:(c + 1) * RS] = yq[0:RS].astype(np.float32) * scales[:, None]
    out += np.asarray(b_o, np.float32)
    return out.reshape(B, S, DIM)
